# revision 12
# baseline (speedup 1.0000x reference)
"""MDTA-style dense attention (B=2, N=4096+8 summary tokens, C=192, H=8, D=24)
on 8 Trainium2 NeuronCores.

Sharding: data-parallel over batch B (2) x tensor-parallel over heads
(4 groups of 2 heads) -> 8 cores, each core computes attention for one batch
and two heads plus its slice of the qkv projection. The output projection,
softmax normalization (divide by denominator) and the cross-strip /
cross-head reductions are done on the host: each core returns raw
PV partial sums (4 key-strips x [ones-denominator row + 24 v dims]) in fp32.

Device algorithm per core (all layouts transposed: feature-major):
  - qkv projection: q,k in [d, n] layout; v in [m, d] layout (V_aug with an
    appended ones column so the PV matmul also yields the softmax
    denominator).
  - S^T blocks = k_blk^T q (contraction over d on partitions, 4-way
    row-tiled on the PE), giving S in fp32 PSUM.
  - exp is split across two engines to break the ScalarE bottleneck:
    6 of every 11 key-block groups use ScalarE's exact exp; 5 use a
    VectorE integer-trick approximation exp(x) ~= bitcast_bf16(int16(
    x * 2^7/ln2 + B)) (Schraudolph at bf16 precision), accurate to ~4%
    per element which washes out in the softmax ratio.
  - PV runs as full-array (unmasked) matmuls with zero-padded [128,128]
    per-head weights: block mb's strip lands at output partitions
    32*(mb%4)..+32, zeros elsewhere. Unmasked matmuls keep the PE clock
    warm (2.4 GHz) -- tile_position-masked matmuls do not register with
    the PE activity monitor.
Keys are zero-padded 4104 -> 4224 (33*128); padded keys produce S=0 ->
e ~= 1 but multiply V_aug rows that are zero, contributing nothing.
"""

import numpy as np

import concourse.bass as bass
import concourse.tile as tile
from concourse import bacc, mybir
from concourse.bass_utils import run_bass_kernel_spmd

# Problem constants (hardcoded per contract).
B = 2
N = 4096          # output tokens
K_SUM = 8         # summary tokens
NT = N + K_SUM    # 4104 total tokens
NP = 4224         # padded key count = 33 * 128
C = 192
H = 8
D = 24
NCORES = 8

CI = 512          # query chunk (8 chunks over 4096)
MB = 128          # key block
GROUP = 3         # key blocks per exp group (3 PSUM banks)
NCHUNKS = N // CI            # 8
MBLOCKS = NP // MB           # 33
NGROUPS = (MBLOCKS + GROUP - 1) // GROUP  # 11
DVE_GROUPS = frozenset({1, 3, 5, 7, 9})   # groups exp'd on VectorE

# Schraudolph integer exp at bf16 precision:
# exp(x) ~= bitcast_bf16(int16(x * 2^7/ln2 + (127*2^7 - 0.035248*2^7)))
EXP_SCALE = float(np.float32(2.0 ** 7 / np.log(2.0)))
EXP_BIAS = float(np.float32(127.0 * 128.0 - 0.035248 * 128.0))

F32 = mybir.dt.float32
F32R = mybir.dt.float32r
I16 = mybir.dt.int16
F16 = mybir.dt.float16
BF16 = mybir.dt.bfloat16

_CACHED = {}


def build_program():
    nc = bacc.Bacc("TRN2", target_bir_lowering=False, debug=False,
                   num_devices=NCORES)
    xt_d = nc.dram_tensor("XT", [C + 1, NP], F16, kind="ExternalInput")
    wt_d = nc.dram_tensor("WT", [C + 1, 160], F16, kind="ExternalInput")
    out_d = nc.dram_tensor("OT", [2 * MB, N], F32, kind="ExternalOutput")

    with tile.TileContext(nc) as tc:
        with tc.tile_pool(name="singles", bufs=1) as singles:
            xt0 = singles.tile([128, NP], F16, tag="xt0")
            xt1 = singles.tile([65, NP], F16, tag="xt1")
            wt0 = singles.tile([128, 160], F16, tag="wt0")
            wt1 = singles.tile([65, 160], F16, tag="wt1")
            qks = singles.tile([128, NP], F16, tag="qks")
            # 4-strip replicas for row-tiled S matmuls (K=24 uses only a
            # 32-row slice of the PE array; 4 strips run concurrently)
            q4 = [singles.tile([128, N], F16, tag=f"q4_{h}", name=f"q4_{h}")
                  for h in range(2)]
            k4 = [singles.tile([128, 9 * MB], F16, tag=f"k4_{h}",
                               name=f"k4_{h}") for h in range(2)]
            # per-head PV weights, zero-padded to full M=128: block mb's
            # [ones, v(24), 0*7] strip sits at columns 32*(mb%4)..+32 and
            # zeros elsewhere, so the full-array (unmasked) PV matmul
            # accumulates each block into its own 32-partition slice while
            # adding zeros to the rest. Unmasked matmuls keep the PE
            # activity monitor warm (2.4 GHz); tile_position-masked ones
            # do not register and the PE decays to 1.2 GHz.
            vaug = [singles.tile([128, MBLOCKS, 128], BF16, tag=f"vaug{h}",
                                 name=f"vaug{h}") for h in range(2)]
            nc.vector.memset(vaug[0][:], 0.0)
            nc.vector.memset(vaug[1][:], 0.0)
            # zero operands for the PSUM-opening dummy matmul and the
            # full-array HAM heartbeats (masked/tiled matmuls do not assert
            # the PE activity monitor, so without these the PE stays
            # clock-gated at 1.2 GHz)
            zw = singles.tile([128, 128], F16, tag="zw")
            zx = singles.tile([128, CI], F16, tag="zx")
            nc.vector.memset(zw[:], 0.0)
            nc.vector.memset(zx[:], 0.0)

            # Input loads (host supplies fp16). Weights first: the first
            # production matmul needs them, so they must not queue behind
            # the large XT transfers. XT is chunked so compute starts early.
            nc.sync.dma_start(out=wt0[:], in_=wt_d[0:128, :])
            nc.sync.dma_start(out=wt1[:], in_=wt_d[128:193, :])
            for c0 in range(0, NP, 1056):
                nc.sync.dma_start(out=xt0[:, c0:c0 + 1056],
                                  in_=xt_d[0:128, c0:c0 + 1056])
                nc.gpsimd.dma_start(out=xt1[:, c0:c0 + 1056],
                                    in_=xt_d[128:193, c0:c0 + 1056])

            xts = (xt0, xt1)
            wts = (wt0, wt1)

            # ---- q/k production: 4 roles col-tiled into one PSUM bank,
            # running concurrently on separate 32-column PE strips. Role r
            # lands at partitions 32r, matching the strip layout directly.
            with tc.tile_pool(name="qkpsum", bufs=4, space="PSUM") as qkp:
                for ci in range(9):
                    c0 = ci * CI
                    w = CI if ci < 8 else MB   # last chunk: cols 4096:4224
                    ps = qkp.tile([128, CI], F32, tag="qk")
                    for r in range(4):         # q_h0, q_h1, k_h0, k_h1
                        if ci == 8 and r < 2:
                            continue  # q only needs 4096 cols
                        for kc in range(2):
                            nc.tensor.matmul(
                                ps[32 * r:32 * r + D, :w],
                                lhsT=wts[kc][:, 24 * r:24 * r + D],
                                rhs=xts[kc][:, c0:c0 + w],
                                start=(kc == 0), stop=(kc == 1),
                                tile_position=(0, 32 * r),
                                skip_group_check=True)
                    nc.vector.tensor_copy(out=qks[:, c0:c0 + w],
                                          in_=ps[:, :w])

            # replicate q into 4 partition strips; scatter k blocks
            # round-robin over strips (block mb -> strip mb%4, col mb//4).
            # One large DMA per (head, strip) — many small DMAs cost ~700ns
            # each in queue time and delay the attention start by ~30us.
            # strips whose partition range already matches qks read it
            # directly in the S matmuls (no replication DMA needed):
            # q for strip h, k for strip 2+h.
            engs = [nc.sync, nc.gpsimd, nc.scalar]
            ei = 0
            for h in range(2):
                q_src = qks[32 * h:32 * h + D, :]
                k_src = qks[64 + 32 * h:64 + 32 * h + D, :]
                for st in range(4):
                    if st != h:
                        engs[ei % 3].dma_start(
                            out=q4[h][32 * st:32 * st + D, :],
                            in_=q_src[:, 0:N])
                        ei += 1
                    if st != 2 + h:
                        nblk = 9 if st == 0 else 8
                        src_v = k_src[:, 0:MBLOCKS * MB].rearrange(
                            "p (t c) -> p t c", c=MB)[:, st::4, :]
                        dst_v = k4[h][32 * st:32 * st + D,
                                      0:nblk * MB].rearrange(
                            "p (t c) -> p t c", c=MB)
                        engs[ei % 3].dma_start(out=dst_v, in_=src_v)
                        ei += 1

            # ---- V_aug production: [m, d] layout via per-block matmuls ----
            with tc.tile_pool(name="vpsum", bufs=4, space="PSUM") as vps:
                # V_aug per-head 32-col strip: [ones-indicator, v (24), 0*7].
                # The indicator feature row of XT makes the matmul emit the
                # ones column (and zeros for padded keys) directly.
                for mb in range(MBLOCKS):
                    m0 = mb * MB
                    ps = vps.tile([128, 64], F32, tag="v")
                    for kc in range(2):
                        nc.tensor.matmul(
                            ps[:],
                            lhsT=xts[kc][:, m0:m0 + MB],
                            rhs=wts[kc][:, 96:160],
                            start=(kc == 0), stop=(kc == 1))
                    st = 32 * (mb % 4)
                    nc.vector.tensor_copy(
                        out=vaug[0][:, mb, st:st + 32], in_=ps[:, 0:32])
                    nc.vector.tensor_copy(
                        out=vaug[1][:, mb, st:st + 32], in_=ps[:, 32:64])
                # PE warmup: dense full-array matmuls right after V
                # production. The PE clock gate (HAM) unthrottles to
                # 2.4 GHz only after a ~3.4us window of dense activity and
                # re-throttles after a ~3.4us idle window; this burst
                # bridges the production->attention transition so the PE
                # enters attention warm, and attention's own gaps (<2us)
                # then hold it warm.
                for w in range(20):
                    w_ps = vps.tile([128, CI], F32, tag="wv")
                    nc.tensor.matmul(w_ps[:, :], lhsT=zw[:, :], rhs=zx[:, :],
                                     start=True, stop=True,
                                     skip_group_check=True)

            # ---- attention ----
            # Flat software pipeline over all (ci, h, g) work items:
            # S runs two groups ahead of PV, and exp is split into an
            # [128,2,512] + [128,1,512] call pair so the S psum buffers
            # free incrementally -- the two exp engines (ScalarE exact,
            # VectorE integer-trick) then run saturated back-to-back
            # without waiting on the PE's in-order queue.
            with (tc.tile_pool(name="spsumA", bufs=2, space="PSUM") as spA,
                  tc.tile_pool(name="spsumB", bufs=2, space="PSUM") as spB,
                  tc.tile_pool(name="opsum", bufs=2, space="PSUM") as op,
                  tc.tile_pool(name="expfA", bufs=3) as epfA,
                  tc.tile_pool(name="expfB", bufs=3) as epfB,
                  tc.tile_pool(name="expiA", bufs=3) as epiA,
                  tc.tile_pool(name="expiB", bufs=3) as epiB,
                  tc.tile_pool(name="osb", bufs=3) as ob):
                state = {"o": None}

                def emit_s(ci, h, g):
                    c0 = ci * CI
                    sA = spA.tile([128, 2, CI], F32, tag="sa", name="sA")
                    sB = spB.tile([128, 1, CI], F32, tag="sb", name="sB")
                    for j in range(GROUP):
                        mb = g * GROUP + j
                        st, t = mb % 4, mb // 4
                        p0 = 32 * st
                        dst = sA[:, j, :] if j < 2 else sB[:, 0, :]
                        if st == 2 + h:
                            k_ap = qks[64 + 32 * h:64 + 32 * h + D,
                                       mb * MB:(mb + 1) * MB]
                        else:
                            k_ap = k4[h][p0:p0 + D, t * MB:(t + 1) * MB]
                        if st == h:
                            q_ap = qks[32 * h:32 * h + D, c0:c0 + CI]
                        else:
                            q_ap = q4[h][p0:p0 + D, c0:c0 + CI]
                        nc.tensor.matmul(
                            dst,
                            lhsT=k_ap,
                            rhs=q_ap,
                            start=True, stop=True,
                            tile_position=(p0, 0))
                    return sA, sB

                def emit_exp(g, sA, sB):
                    if g % 2 == 1:      # VectorE integer-trick exp
                        eA = epiA.tile([128, 2, CI], I16, tag="eia", name="eiA")
                        eB = epiB.tile([128, 1, CI], I16, tag="eib", name="eiB")
                        for s_t, e_t in ((sA, eA), (sB, eB)):
                            nc.vector.tensor_scalar(
                                out=e_t[:], in0=s_t[:],
                                scalar1=EXP_SCALE, scalar2=EXP_BIAS,
                                op0=mybir.AluOpType.mult,
                                op1=mybir.AluOpType.add)
                    else:               # ScalarE exact exp
                        eA = epfA.tile([128, 2, CI], BF16, tag="efa", name="efA")
                        eB = epfB.tile([128, 1, CI], BF16, tag="efb", name="efB")
                        for s_t, e_t in ((sA, eA), (sB, eB)):
                            nc.scalar.activation(
                                out=e_t[:], in_=s_t[:],
                                func=mybir.ActivationFunctionType.Exp)
                    return eA, eB

                def emit_pv(ci, h, g, eA, eB):
                    c0 = ci * CI
                    if g == 0:
                        state["o"] = op.tile([128, CI], F32, tag="o", name="o_ps")
                    o_ps = state["o"]
                    for j in range(GROUP):
                        mb = g * GROUP + j
                        e_sl = eA[:, j, :] if j < 2 else eB[:, 0, :]
                        nc.tensor.matmul(
                            o_ps[:, :],
                            lhsT=vaug[h][:, mb, :],
                            rhs=(e_sl.bitcast(BF16) if g % 2 == 1 else e_sl),
                            start=(mb == 0),
                            stop=(mb == MBLOCKS - 1))
                    if g == NGROUPS - 1:
                        # raw partials out; host reduces strips, divides by
                        # the denominator rows and applies Wout. Alternate
                        # the PSUM->SBUF copy engine to balance load.
                        o_sb = ob.tile([128, CI], F32, tag="osb", name="o_sb")
                        if (ci + h) % 2 == 0:
                            nc.scalar.activation(
                                out=o_sb[:], in_=o_ps[:],
                                func=mybir.ActivationFunctionType.Copy)
                        else:
                            nc.vector.tensor_copy(out=o_sb[:], in_=o_ps[:])
                        nc.sync.dma_start(
                            out=out_d[128 * h:128 * (h + 1), c0:c0 + CI],
                            in_=o_sb[:])

                flat = [(ci, h, g) for ci in range(NCHUNKS)
                        for h in range(2) for g in range(NGROUPS)]
                pend = []
                for it in flat:
                    sA, sB = emit_s(*it)
                    e_pair = emit_exp(it[2], sA, sB)
                    pend.append((it, e_pair))
                    if len(pend) > 2:
                        pit, pe = pend.pop(0)
                        emit_pv(*pit, *pe)
                for pit, pe in pend:
                    emit_pv(*pit, *pe)

    nc.compile()
    return nc


def make_in_maps(X_flat, S_tokens, Wqkv, Wout, temperature):
    temp = np.asarray(temperature, dtype=np.float32).reshape(H)
    Wq = np.asarray(Wqkv[0:C], dtype=np.float32)
    Wk = np.asarray(Wqkv[C:2 * C], dtype=np.float32)
    Wv = np.asarray(Wqkv[2 * C:3 * C], dtype=np.float32)

    xts = []
    for b in range(B):
        x_in = np.concatenate([np.asarray(X_flat[b], dtype=np.float32),
                               np.asarray(S_tokens[b], dtype=np.float32)], axis=0)
        xt = np.zeros((C + 1, NP), dtype=np.float32)
        xt[:C, :NT] = np.ascontiguousarray(x_in.T)
        xt[C, :NT] = 1.0  # indicator feature -> ones column of V_aug
        xts.append(xt)

    in_maps = []
    for core in range(NCORES):
        b = core // 4
        h0 = 2 * (core % 4)
        h1 = h0 + 1
        wt = np.zeros((C + 1, 160), dtype=np.float32)
        wt[:C, 0:24] = (Wq[h0 * D:(h0 + 1) * D] * temp[h0]).T
        wt[:C, 24:48] = (Wq[h1 * D:(h1 + 1) * D] * temp[h1]).T
        wt[:C, 48:72] = Wk[h0 * D:(h0 + 1) * D].T
        wt[:C, 72:96] = Wk[h1 * D:(h1 + 1) * D].T
        wt[C, 96] = 1.0                                   # ones indicator h0
        wt[:C, 97:121] = Wv[h0 * D:(h0 + 1) * D].T
        wt[C, 128] = 1.0                                  # ones indicator h1
        wt[:C, 129:153] = Wv[h1 * D:(h1 + 1) * D].T
        in_maps.append({
            "XT": np.ascontiguousarray(xts[b]).astype(np.float16),
            "WT": np.ascontiguousarray(wt).astype(np.float16),
        })
    return in_maps


def run(in_maps, **kwargs):
    if "nc" not in _CACHED:
        _CACHED["nc"] = build_program()
    return run_bass_kernel_spmd(_CACHED["nc"], in_maps,
                                core_ids=list(range(NCORES)), **kwargs)


def unshard(results, Wout):
    """Host-side finish: reduce the 4 key strips, normalize by the softmax
    denominator (strip row 0) and apply the output projection per head."""
    Wout = np.asarray(Wout, dtype=np.float32)
    out = np.zeros((B, N, C), dtype=np.float32)
    for core in range(NCORES):
        b = core // 4
        ot = np.asarray(results[core]["OT"], dtype=np.float32)
        for hl in range(2):
            hg = 2 * (core % 4) + hl
            blk = ot[128 * hl:128 * (hl + 1)].reshape(4, 32, N)
            den = blk[:, 0, :].sum(axis=0)          # [N]
            o24 = blk[:, 1:25, :].sum(axis=0)       # [24, N]
            attn = o24 / den[None, :]
            out[b] += (Wout[:, hg * D:(hg + 1) * D] @ attn).T
    return out


def kernel(X_flat, S_tokens, Wqkv, Wout, temperature):
    in_maps = make_in_maps(X_flat, S_tokens, Wqkv, Wout, temperature)
    res = run(in_maps)
    return unshard(res.results, Wout)


# revision 13
# speedup vs baseline: 1.0268x; 1.0268x over previous
"""MDTA-style dense attention (B=2, N=4096+8 summary tokens, C=192, H=8, D=24)
on 8 Trainium2 NeuronCores.

Sharding: data-parallel over batch B (2) x tensor-parallel over heads
(4 groups of 2 heads) -> 8 cores, each core computes attention for one batch
and two heads plus its slice of the qkv projection. The output projection,
softmax normalization (divide by denominator) and the cross-strip /
cross-head reductions are done on the host: each core returns raw
PV partial sums (4 key-strips x [ones-denominator row + 24 v dims]) in fp32.

Device algorithm per core (all layouts transposed: feature-major):
  - qkv projection: q,k in [d, n] layout; v in [m, d] layout (V_aug with an
    appended ones column so the PV matmul also yields the softmax
    denominator).
  - S^T blocks = k_blk^T q (contraction over d on partitions, 4-way
    row-tiled on the PE), giving S in fp32 PSUM.
  - exp is split across two engines to break the ScalarE bottleneck:
    ~half the key-block groups use ScalarE's exact exp; the rest use a
    VectorE integer-trick approximation exp(x) ~= bitcast_bf16(int16(
    x * 2^7/ln2 + B)) (Schraudolph at bf16 precision), accurate to ~4%
    per element which washes out in the softmax ratio.
  - PV runs as full-array (unmasked) matmuls with zero-padded [128,128]
    per-head weights: block mb's strip lands at output partitions
    32*(mb%4)..+32, zeros elsewhere. Unmasked matmuls keep the PE clock
    warm (2.4 GHz) -- tile_position-masked matmuls do not register with
    the PE activity monitor.
Keys are zero-padded 4104 -> 4224 (33*128); padded keys produce S=0 ->
e ~= 1 but multiply V_aug rows that are zero, contributing nothing.
"""

import numpy as np

import concourse.bass as bass
import concourse.tile as tile
from concourse import bacc, mybir
from concourse.bass_utils import run_bass_kernel_spmd

# Problem constants (hardcoded per contract).
B = 2
N = 4096          # output tokens
K_SUM = 8         # summary tokens
NT = N + K_SUM    # 4104 total tokens
NP = 4224         # padded key count = 33 * 128
C = 192
H = 8
D = 24
NCORES = 8

CI = 512          # query chunk (8 chunks over 4096)
MB = 128          # key block
GROUP = 3         # key blocks per exp group (3 PSUM banks)
NCHUNKS = N // CI            # 8
MBLOCKS = NP // MB           # 33
NGROUPS = (MBLOCKS + GROUP - 1) // GROUP  # 11
DVE_GROUPS = frozenset({1, 3, 5, 7, 9})   # groups exp'd on VectorE

# Schraudolph integer exp at bf16 precision:
# exp(x) ~= bitcast_bf16(int16(x * 2^7/ln2 + (127*2^7 - 0.035248*2^7)))
EXP_SCALE = float(np.float32(2.0 ** 7 / np.log(2.0)))
EXP_BIAS = float(np.float32(127.0 * 128.0 - 0.035248 * 128.0))

F32 = mybir.dt.float32
F32R = mybir.dt.float32r
I16 = mybir.dt.int16
F16 = mybir.dt.float16
BF16 = mybir.dt.bfloat16

_CACHED = {}


def build_program():
    nc = bacc.Bacc("TRN2", target_bir_lowering=False, debug=False,
                   num_devices=NCORES)
    xt_d = nc.dram_tensor("XT", [C + 1, NP], F16, kind="ExternalInput")
    wt_d = nc.dram_tensor("WT", [C + 1, 160], F16, kind="ExternalInput")
    out_d = nc.dram_tensor("OT", [2 * MB, N], F32, kind="ExternalOutput")

    with tile.TileContext(nc) as tc:
        with tc.tile_pool(name="singles", bufs=1) as singles:
            xt0 = singles.tile([128, NP], F16, tag="xt0")
            xt1 = singles.tile([65, NP], F16, tag="xt1")
            wt0 = singles.tile([128, 160], F16, tag="wt0")
            wt1 = singles.tile([65, 160], F16, tag="wt1")
            qks = singles.tile([128, NP], F16, tag="qks")
            # 4-strip replicas for row-tiled S matmuls (K=24 uses only a
            # 32-row slice of the PE array; 4 strips run concurrently)
            q4 = [singles.tile([128, N], F16, tag=f"q4_{h}", name=f"q4_{h}")
                  for h in range(2)]
            k4 = [singles.tile([128, 9 * MB], F16, tag=f"k4_{h}",
                               name=f"k4_{h}") for h in range(2)]
            # per-head PV weights, zero-padded to full M=128: block mb's
            # [ones, v(24), 0*7] strip sits at columns 32*(mb%4)..+32 and
            # zeros elsewhere, so the full-array (unmasked) PV matmul
            # accumulates each block into its own 32-partition slice while
            # adding zeros to the rest. Unmasked matmuls keep the PE
            # activity monitor warm (2.4 GHz); tile_position-masked ones
            # do not register and the PE decays to 1.2 GHz.
            vaug = [singles.tile([128, MBLOCKS, 128], BF16, tag=f"vaug{h}",
                                 name=f"vaug{h}") for h in range(2)]
            nc.vector.memset(vaug[0][:], 0.0)
            nc.vector.memset(vaug[1][:], 0.0)
            # zero operands for the PSUM-opening dummy matmul and the
            # full-array HAM heartbeats (masked/tiled matmuls do not assert
            # the PE activity monitor, so without these the PE stays
            # clock-gated at 1.2 GHz)
            zw = singles.tile([128, 128], F16, tag="zw")
            zx = singles.tile([128, CI], F16, tag="zx")
            nc.vector.memset(zw[:], 0.0)
            nc.vector.memset(zx[:], 0.0)

            # Input loads (host supplies fp16). Weights first: the first
            # production matmul needs them, so they must not queue behind
            # the large XT transfers. XT is chunked so compute starts early.
            nc.sync.dma_start(out=wt0[:], in_=wt_d[0:128, :])
            nc.sync.dma_start(out=wt1[:], in_=wt_d[128:193, :])
            for c0 in range(0, NP, 1056):
                nc.sync.dma_start(out=xt0[:, c0:c0 + 1056],
                                  in_=xt_d[0:128, c0:c0 + 1056])
                nc.gpsimd.dma_start(out=xt1[:, c0:c0 + 1056],
                                    in_=xt_d[128:193, c0:c0 + 1056])

            xts = (xt0, xt1)
            wts = (wt0, wt1)

            # ---- q/k production: 4 roles col-tiled into one PSUM bank,
            # running concurrently on separate 32-column PE strips. Role r
            # lands at partitions 32r, matching the strip layout directly.
            with tc.tile_pool(name="qkpsum", bufs=4, space="PSUM") as qkp:
                for ci in range(9):
                    c0 = ci * CI
                    w = CI if ci < 8 else MB   # last chunk: cols 4096:4224
                    ps = qkp.tile([128, CI], F32, tag="qk")
                    for r in range(4):         # q_h0, q_h1, k_h0, k_h1
                        if ci == 8 and r < 2:
                            continue  # q only needs 4096 cols
                        for kc in range(2):
                            nc.tensor.matmul(
                                ps[32 * r:32 * r + D, :w],
                                lhsT=wts[kc][:, 24 * r:24 * r + D],
                                rhs=xts[kc][:, c0:c0 + w],
                                start=(kc == 0), stop=(kc == 1),
                                tile_position=(0, 32 * r),
                                skip_group_check=True)
                    nc.vector.tensor_copy(out=qks[:, c0:c0 + w],
                                          in_=ps[:, :w])

            # replicate q into 4 partition strips; scatter k blocks
            # round-robin over strips (block mb -> strip mb%4, col mb//4).
            # One large DMA per (head, strip) — many small DMAs cost ~700ns
            # each in queue time and delay the attention start by ~30us.
            # strips whose partition range already matches qks read it
            # directly in the S matmuls (no replication DMA needed):
            # q for strip h, k for strip 2+h.
            engs = [nc.sync, nc.gpsimd, nc.scalar]
            ei = 0
            for h in range(2):
                q_src = qks[32 * h:32 * h + D, :]
                k_src = qks[64 + 32 * h:64 + 32 * h + D, :]
                for st in range(4):
                    if st != h:
                        engs[ei % 3].dma_start(
                            out=q4[h][32 * st:32 * st + D, :],
                            in_=q_src[:, 0:N])
                        ei += 1
                    if st != 2 + h:
                        nblk = 9 if st == 0 else 8
                        src_v = k_src[:, 0:MBLOCKS * MB].rearrange(
                            "p (t c) -> p t c", c=MB)[:, st::4, :]
                        dst_v = k4[h][32 * st:32 * st + D,
                                      0:nblk * MB].rearrange(
                            "p (t c) -> p t c", c=MB)
                        engs[ei % 3].dma_start(out=dst_v, in_=src_v)
                        ei += 1

            # ---- V_aug production: [m, d] layout via per-block matmuls ----
            with tc.tile_pool(name="vpsum", bufs=4, space="PSUM") as vps:
                # V_aug per-head 32-col strip: [ones-indicator, v (24), 0*7].
                # The indicator feature row of XT makes the matmul emit the
                # ones column (and zeros for padded keys) directly.
                for mb in range(MBLOCKS):
                    m0 = mb * MB
                    ps = vps.tile([128, 64], F32, tag="v")
                    for kc in range(2):
                        nc.tensor.matmul(
                            ps[:],
                            lhsT=xts[kc][:, m0:m0 + MB],
                            rhs=wts[kc][:, 96:160],
                            start=(kc == 0), stop=(kc == 1))
                    st = 32 * (mb % 4)
                    nc.vector.tensor_copy(
                        out=vaug[0][:, mb, st:st + 32], in_=ps[:, 0:32])
                    nc.vector.tensor_copy(
                        out=vaug[1][:, mb, st:st + 32], in_=ps[:, 32:64])
                # PE warmup: dense full-array matmuls right after V
                # production. The PE clock gate (HAM) unthrottles to
                # 2.4 GHz only after a ~3.4us window of dense activity and
                # re-throttles after a ~3.4us idle window; this burst
                # bridges the production->attention transition so the PE
                # enters attention warm, and attention's own gaps (<2us)
                # then hold it warm.
                for w in range(12):
                    w_ps = vps.tile([128, CI], F32, tag="wv")
                    nc.tensor.matmul(w_ps[:, :], lhsT=zw[:, :], rhs=zx[:, :],
                                     start=True, stop=True,
                                     skip_group_check=True)

            # ---- attention ----
            # Flat software pipeline over all (ci, h, g) work items:
            # S runs two groups ahead of PV, and exp is split into an
            # [128,2,512] + [128,1,512] call pair so the S psum buffers
            # free incrementally -- the two exp engines (ScalarE exact,
            # VectorE integer-trick) then run saturated back-to-back
            # without waiting on the PE's in-order queue.
            with (tc.tile_pool(name="spsumA", bufs=2, space="PSUM") as spA,
                  tc.tile_pool(name="spsumB", bufs=2, space="PSUM") as spB,
                  tc.tile_pool(name="opsum", bufs=2, space="PSUM") as op,
                  tc.tile_pool(name="expfA", bufs=3) as epfA,
                  tc.tile_pool(name="expfB", bufs=3) as epfB,
                  tc.tile_pool(name="expiA", bufs=3) as epiA,
                  tc.tile_pool(name="expiB", bufs=3) as epiB,
                  tc.tile_pool(name="osb", bufs=3) as ob):
                state = {"o": None}

                def emit_s(ci, h, g):
                    c0 = ci * CI
                    sA = spA.tile([128, 2, CI], F32, tag="sa", name="sA")
                    sB = spB.tile([128, 1, CI], F32, tag="sb", name="sB")
                    for j in range(GROUP):
                        mb = g * GROUP + j
                        st, t = mb % 4, mb // 4
                        p0 = 32 * st
                        dst = sA[:, j, :] if j < 2 else sB[:, 0, :]
                        if st == 2 + h:
                            k_ap = qks[64 + 32 * h:64 + 32 * h + D,
                                       mb * MB:(mb + 1) * MB]
                        else:
                            k_ap = k4[h][p0:p0 + D, t * MB:(t + 1) * MB]
                        if st == h:
                            q_ap = qks[32 * h:32 * h + D, c0:c0 + CI]
                        else:
                            q_ap = q4[h][p0:p0 + D, c0:c0 + CI]
                        nc.tensor.matmul(
                            dst,
                            lhsT=k_ap,
                            rhs=q_ap,
                            start=True, stop=True,
                            tile_position=(p0, 0))
                    return sA, sB

                def emit_exp(g, sA, sB, par):
                    if (g + par) % 2 == 1:  # VectorE integer-trick exp
                        eA = epiA.tile([128, 2, CI], I16, tag="eia", name="eiA")
                        eB = epiB.tile([128, 1, CI], I16, tag="eib", name="eiB")
                        for s_t, e_t in ((sA, eA), (sB, eB)):
                            nc.vector.tensor_scalar(
                                out=e_t[:], in0=s_t[:],
                                scalar1=EXP_SCALE, scalar2=EXP_BIAS,
                                op0=mybir.AluOpType.mult,
                                op1=mybir.AluOpType.add)
                    else:               # ScalarE exact exp
                        eA = epfA.tile([128, 2, CI], BF16, tag="efa", name="efA")
                        eB = epfB.tile([128, 1, CI], BF16, tag="efb", name="efB")
                        for s_t, e_t in ((sA, eA), (sB, eB)):
                            nc.scalar.activation(
                                out=e_t[:], in_=s_t[:],
                                func=mybir.ActivationFunctionType.Exp)
                    return eA, eB

                def emit_pv(ci, h, g, eA, eB):
                    par = (2 * ci + h) % 2
                    c0 = ci * CI
                    if g == 0:
                        state["o"] = op.tile([128, CI], F32, tag="o", name="o_ps")
                    o_ps = state["o"]
                    for j in range(GROUP):
                        mb = g * GROUP + j
                        e_sl = eA[:, j, :] if j < 2 else eB[:, 0, :]
                        nc.tensor.matmul(
                            o_ps[:, :],
                            lhsT=vaug[h][:, mb, :],
                            rhs=(e_sl.bitcast(BF16)
                                 if (g + par) % 2 == 1 else e_sl),
                            start=(mb == 0),
                            stop=(mb == MBLOCKS - 1))
                    if g == NGROUPS - 1:
                        # raw partials out; host reduces strips, divides by
                        # the denominator rows and applies Wout. Alternate
                        # the PSUM->SBUF copy engine to balance load.
                        o_sb = ob.tile([128, CI], F32, tag="osb", name="o_sb")
                        if (ci + h) % 2 == 0:
                            nc.scalar.activation(
                                out=o_sb[:], in_=o_ps[:],
                                func=mybir.ActivationFunctionType.Copy)
                        else:
                            nc.vector.tensor_copy(out=o_sb[:], in_=o_ps[:])
                        nc.sync.dma_start(
                            out=out_d[128 * h:128 * (h + 1), c0:c0 + CI],
                            in_=o_sb[:])

                flat = [(ci, h, g) for ci in range(NCHUNKS)
                        for h in range(2) for g in range(NGROUPS)]
                pend = []
                for it in flat:
                    sA, sB = emit_s(*it)
                    e_pair = emit_exp(it[2], sA, sB,
                                      (2 * it[0] + it[1]) % 2)
                    pend.append((it, e_pair))
                    if len(pend) > 2:
                        pit, pe = pend.pop(0)
                        emit_pv(*pit, *pe)
                for pit, pe in pend:
                    emit_pv(*pit, *pe)

    nc.compile()
    return nc


def make_in_maps(X_flat, S_tokens, Wqkv, Wout, temperature):
    temp = np.asarray(temperature, dtype=np.float32).reshape(H)
    Wq = np.asarray(Wqkv[0:C], dtype=np.float32)
    Wk = np.asarray(Wqkv[C:2 * C], dtype=np.float32)
    Wv = np.asarray(Wqkv[2 * C:3 * C], dtype=np.float32)

    xts = []
    for b in range(B):
        x_in = np.concatenate([np.asarray(X_flat[b], dtype=np.float32),
                               np.asarray(S_tokens[b], dtype=np.float32)], axis=0)
        xt = np.zeros((C + 1, NP), dtype=np.float32)
        xt[:C, :NT] = np.ascontiguousarray(x_in.T)
        xt[C, :NT] = 1.0  # indicator feature -> ones column of V_aug
        xts.append(xt)

    in_maps = []
    for core in range(NCORES):
        b = core // 4
        h0 = 2 * (core % 4)
        h1 = h0 + 1
        wt = np.zeros((C + 1, 160), dtype=np.float32)
        wt[:C, 0:24] = (Wq[h0 * D:(h0 + 1) * D] * temp[h0]).T
        wt[:C, 24:48] = (Wq[h1 * D:(h1 + 1) * D] * temp[h1]).T
        wt[:C, 48:72] = Wk[h0 * D:(h0 + 1) * D].T
        wt[:C, 72:96] = Wk[h1 * D:(h1 + 1) * D].T
        wt[C, 96] = 1.0                                   # ones indicator h0
        wt[:C, 97:121] = Wv[h0 * D:(h0 + 1) * D].T
        wt[C, 128] = 1.0                                  # ones indicator h1
        wt[:C, 129:153] = Wv[h1 * D:(h1 + 1) * D].T
        in_maps.append({
            "XT": np.ascontiguousarray(xts[b]).astype(np.float16),
            "WT": np.ascontiguousarray(wt).astype(np.float16),
        })
    return in_maps


def run(in_maps, **kwargs):
    if "nc" not in _CACHED:
        _CACHED["nc"] = build_program()
    return run_bass_kernel_spmd(_CACHED["nc"], in_maps,
                                core_ids=list(range(NCORES)), **kwargs)


def unshard(results, Wout):
    """Host-side finish: reduce the 4 key strips, normalize by the softmax
    denominator (strip row 0) and apply the output projection per head."""
    Wout = np.asarray(Wout, dtype=np.float32)
    out = np.zeros((B, N, C), dtype=np.float32)
    for core in range(NCORES):
        b = core // 4
        ot = np.asarray(results[core]["OT"], dtype=np.float32)
        for hl in range(2):
            hg = 2 * (core % 4) + hl
            blk = ot[128 * hl:128 * (hl + 1)].reshape(4, 32, N)
            den = blk[:, 0, :].sum(axis=0)          # [N]
            o24 = blk[:, 1:25, :].sum(axis=0)       # [24, N]
            attn = o24 / den[None, :]
            out[b] += (Wout[:, hg * D:(hg + 1) * D] @ attn).T
    return out


def kernel(X_flat, S_tokens, Wqkv, Wout, temperature):
    in_maps = make_in_maps(X_flat, S_tokens, Wqkv, Wout, temperature)
    res = run(in_maps)
    return unshard(res.results, Wout)
